# revision 20
# baseline (speedup 1.0000x reference)
"""Trainium2 8-core tensor-parallel causal attention layer (prefill, pos=0),
collective-free.

Sharding: heads split across 8 cores (2 heads each). Each core computes QKV
projections + RoPE + causal attention for its 2 heads over all 4 batches,
then a PARTIAL output projection (its 256 rows of Wo's input dim, FULL output
dim D). The 8 partial outputs are summed host-side -- no on-device collective.

Per core:
  1. Q^T/K^T (head-dim-major) and V (token-major) projections from a
     host-relayouted bf16 copy of h (one contiguous 2MB block per 512-token
     tile -> ONE DMA per tile, double-buffered across tiles),
  2. RoPE via an even/odd head-dim permutation baked into Wq/Wk columns;
     the four projection chains of a token tile share one [128, 2048] bf16
     staging tile so the partition-half swap is 2 batched SBUF DMAs,
  3. causal attention in the transposed domain (scores^T = K^T_tile.T @ Q^T)
     with exact causal trim; exp on ACT; row sums via parity-partial DVE
     accumulation + one all-ones matmul (contract + partition-broadcast);
     reciprocal on DVE; normalization fused into the PSUM->SBUF copy,
  4. partial Wo blocks copied PSUM->SBUF in groups of 4 m-blocks and stored
     with ONE DMA per group to a group-contiguous DRAM layout.
Schedule: attention of batch b is interleaved under projection of batch b
(paced by tile readiness); leftover groups + Wo blocks fill the next batch's
projection.  Host-side: inputs transposed/sliced/cast bf16; partials summed.
"""

import numpy as np
import ml_dtypes

import concourse.bass as bass
import concourse.tile as tile
from concourse import bacc, mybir
from concourse.bass_utils import run_bass_kernel_spmd

BF16 = mybir.dt.bfloat16
F32 = mybir.dt.float32
AF = mybir.ActivationFunctionType
ALU = mybir.AluOpType

B, S, D = 4, 2048, 2048
H, HD = 16, 128
NCORES = 8
HL = H // NCORES          # heads per core = 2
E = HL * HD               # per-core qkv width = 256
T = B * S                 # tokens = 8192
TT = 512                  # token tile (free dim)
NT = T // TT              # global token tiles = 16
NT_B = S // TT            # token tiles per batch = 4
DC = D // 128             # contraction chunks = 16
SCALE = 1.0 / np.sqrt(HD)

_cache = {}


def _build():
    nc = bacc.Bacc("TRN2", target_bir_lowering=False, debug=False,
                   num_devices=NCORES)

    hB_ext = nc.dram_tensor("hB", [NT, 128, DC * TT], BF16,
                            kind="ExternalInput")
    wq_ext = nc.dram_tensor("wqB", [128, DC * E], BF16, kind="ExternalInput")
    wk_ext = nc.dram_tensor("wkB", [128, DC * E], BF16, kind="ExternalInput")
    wv_ext = nc.dram_tensor("wvB", [128, DC * E], BF16, kind="ExternalInput")
    wo_ext = nc.dram_tensor("woB", [128, HL * D], BF16, kind="ExternalInput")
    cos_ext = nc.dram_tensor("cosT", [128, S], BF16, kind="ExternalInput")
    sin_ext = nc.dram_tensor("sinT", [128, S], BF16, kind="ExternalInput")
    mask_ext = nc.dram_tensor("mask128", [128, 128], BF16,
                              kind="ExternalInput")
    out_ext = nc.dram_tensor("out", [16, 4, 128, 4 * TT], BF16,
                             kind="ExternalOutput")

    with tile.TileContext(nc) as tc:
        with (
            tc.tile_pool(name="weights", bufs=1) as wpool,
            tc.tile_pool(name="consts", bufs=1) as cpool,
            tc.tile_pool(name="ht", bufs=2) as htpool,
            tc.tile_pool(name="qkv", bufs=2) as qkvpool,
            tc.tile_pool(name="attn", bufs=2) as apool,
            tc.tile_pool(name="probs", bufs=7) as prpool,
            tc.tile_pool(name="ssum", bufs=3) as spool,
            tc.tile_pool(name="norm", bufs=3) as npool,
            tc.tile_pool(name="rope", bufs=2) as rbpool,
            tc.tile_pool(name="rtmp", bufs=4) as rpool,
            tc.tile_pool(name="ost", bufs=3) as ostpool,
            tc.tile_pool(name="ps", bufs=5, space="PSUM") as pspool,
            tc.tile_pool(name="psa", bufs=2, space="PSUM") as apspool,
            tc.tile_pool(name="psr", bufs=1, space="PSUM") as rpspool,
        ):
            htb = {}

            def load_ht(gt, chunks=1):
                t = htpool.tile([128, DC * TT], BF16, tag="ht",
                                name=f"ht{gt}")
                w = DC * TT // chunks
                for c in range(chunks):
                    nc.sync.dma_start(t[:, c * w:(c + 1) * w],
                                      hB_ext.ap()[gt][:, c * w:(c + 1) * w])
                htb[gt] = t

            # startup: first two token tiles on the sync queue (first one
            # chunked so the first matmul can start early); weights ordered
            # by first use on the gpsimd queue.
            # startup DMA order. sync ring (HWDGE, FIFO): the first chain's
            # weights, then the first token tile in 4 chunks, then the RoPE
            # tables (needed by ~25us), then the second token tile. gpsimd
            # ring concurrently: remaining Q/K weights, wv, mask, wo.
            HW = DC * HD          # per-head weight width = 2048
            wq_sb = wpool.tile([128, DC * E], BF16, tag="wq", name="wq")
            wk_sb = wpool.tile([128, DC * E], BF16, tag="wk", name="wk")
            nc.sync.dma_start(wq_sb[:, 0:HW], wq_ext.ap()[:, 0:HW])
            load_ht(0, chunks=4)
            cos_sb = cpool.tile([128, S], BF16, tag="cos", name="cos")
            nc.sync.dma_start(cos_sb[:], cos_ext.ap())
            sin_sb = cpool.tile([128, S], BF16, tag="sin", name="sin")
            nc.sync.dma_start(sin_sb[:], sin_ext.ap())
            load_ht(1)
            nc.gpsimd.dma_start(wq_sb[:, HW:2 * HW], wq_ext.ap()[:, HW:2 * HW])
            for lh in range(HL):
                nc.gpsimd.dma_start(
                    wk_sb[:, lh * HW:(lh + 1) * HW],
                    wk_ext.ap()[:, lh * HW:(lh + 1) * HW])
            wv_sb = wpool.tile([128, DC * E], BF16, tag="wv", name="wv")
            nc.gpsimd.dma_start(wv_sb[:], wv_ext.ap())
            mask_sb = cpool.tile([128, 128], BF16, tag="mask", name="mask")
            nc.gpsimd.dma_start(mask_sb[:], mask_ext.ap())
            wo_sb = wpool.tile([128, HL * D], BF16, tag="wo", name="wo")
            nc.gpsimd.dma_start(wo_sb[:], wo_ext.ap())
            ones_sb = cpool.tile([128, 128], BF16, tag="ones", name="ones")
            nc.vector.memset(ones_sb[:], 1.0)
            # HAM pre-warm: ~6us of dummy matmuls while the startup DMAs
            # land, so the real chains start at K=8/8 instead of paying the
            # half-clock window.
            warm_ps = rpspool.tile([128, TT], F32, tag="rps", name="warm")
            for _ in range(72):
                nc.tensor.matmul(warm_ps[:, 0:128], lhsT=ones_sb[:],
                                 rhs=ones_sb[:], start=True, stop=True,
                                 skip_group_check=True)

            # normalized attention outputs for this core's 2 heads, per batch
            attn_sb = {}

            qkv = {}

            def proj(b, pump=lambda n: None):
                """QKV projections + RoPE for batch b; yields after each
                chain / v-tile so attention can interleave (8 yields/tile)."""
                qT = [qkvpool.tile([HD, S], BF16, tag=f"qT{lh}",
                                   name=f"qT{lh}_{b}") for lh in range(HL)]
                kT = [qkvpool.tile([HD, S], BF16, tag=f"kT{lh}",
                                   name=f"kT{lh}_{b}") for lh in range(HL)]
                v_sb = [qkvpool.tile([128, E], BF16, tag=f"v{vt}",
                                     name=f"v{vt}_{b}")
                        for vt in range(S // 128)]
                qkv[b] = (qT, kT, v_sb)
                for tt in range(NT_B):
                    gt = NT_B * b + tt
                    ht = htb.pop(gt)
                    cs = cos_sb[:, tt * TT:(tt + 1) * TT]
                    sn = sin_sb[:, tt * TT:(tt + 1) * TT]
                    qsb = rbpool.tile([128, 4 * TT], BF16, tag="qsb",
                                      name=f"qsb{gt}")
                    qsw = rbpool.tile([128, 4 * TT], BF16, tag="qsw",
                                      name=f"qsw{gt}")
                    chains = [(wq_sb, qT, 0), (wq_sb, qT, 1),
                              (wk_sb, kT, 0), (wk_sb, kT, 1)]
                    for ci, (w_sb, dstT, lh) in enumerate(chains):
                        ps = pspool.tile([128, TT], F32, tag="ps",
                                         name=f"psp{gt}_{ci}")
                        for dc in range(DC):
                            nc.tensor.matmul(
                                ps[:],
                                lhsT=w_sb[:, lh * (DC * HD) + dc * HD:
                                          lh * (DC * HD) + (dc + 1) * HD],
                                rhs=ht[:, dc * TT:(dc + 1) * TT],
                                start=(dc == 0), stop=(dc == DC - 1))
                        # RoPE staging: psum rows 0:64 = even pairs (x0),
                        # 64:128 = odd (x1); ACT-copy PSUM->bf16.
                        nc.scalar.copy(qsb[:, ci * TT:(ci + 1) * TT], ps[:])
                        pump(1)
                        yield
                    # one batched partition-half swap for all 4 chains
                    nc.sync.dma_start(qsw[0:64, :], qsb[64:128, :])
                    nc.sync.dma_start(qsw[64:128, :], qsb[0:64, :])
                    # prefetch next token tile (slot of gt-1 just freed)
                    if gt + 2 < NT and gt + 2 not in htb:
                        load_ht(gt + 2)
                    # RoPE: m1 muls first (independent of the swap DMA) so
                    # the DVE queue isn't head-blocked on DMA latency.
                    m1s = []
                    for ci in range(4):
                        m1 = rpool.tile([128, TT], BF16, tag=f"m1_{ci % 2}",
                                        name=f"m1_{gt}{ci}")
                        nc.vector.tensor_mul(m1[:],
                                             qsb[:, ci * TT:(ci + 1) * TT],
                                             cs)
                        m1s.append(m1)
                    for ci, (w_sb, dstT, lh) in enumerate(chains):
                        dst = dstT[lh][:, tt * TT:(tt + 1) * TT]
                        m1 = m1s[ci]
                        m2 = rpool.tile([128, TT], BF16, tag=f"m2_{ci % 2}",
                                        name=f"m2_{gt}{ci}")
                        nc.vector.tensor_mul(m2[:],
                                             qsw[:, ci * TT:(ci + 1) * TT],
                                             sn)
                        nc.vector.tensor_sub(dst[0:64, :], m1[0:64, :],
                                             m2[0:64, :])
                        nc.vector.tensor_add(dst[64:128, :],
                                             m1[64:128, :], m2[64:128, :])
                    for vt in range(TT // 128):
                        ps = pspool.tile([128, E], F32, tag="ps",
                                         name=f"psv{gt}_{vt}")
                        for dc in range(DC):
                            nc.tensor.matmul(
                                ps[:],
                                lhsT=ht[:, dc * TT + vt * 128:
                                        dc * TT + (vt + 1) * 128],
                                rhs=wv_sb[:, dc * E:(dc + 1) * E],
                                start=(dc == 0), stop=(dc == DC - 1))
                        nc.scalar.copy(v_sb[tt * 4 + vt][:], ps[:])
                        pump(1)
                        yield

            def attn_groups(b, qT, kT, v_sb, pump=lambda n: None):
                """qt-outer/head-inner; yields (qt, lh) after each group so
                Wo blocks for tile qt can start once both heads are done."""
                at = [apool.tile([128, S], BF16, tag=f"at{lh}",
                                 name=f"at{lh}_{b}") for lh in range(HL)]
                attn_sb[b] = at
                for qt in range(NT_B):
                    for lh in range(HL):
                        n_kt = 4 * (qt + 1)
                        Sa = spool.tile([128, TT], BF16, tag="Sa",
                                        name=f"Sa{b}{lh}{qt}")
                        Sb = spool.tile([128, TT], BF16, tag="Sb",
                                        name=f"Sb{b}{lh}{qt}")
                        if qt == 0:
                            nc.vector.memset(Sb[:, 0:128], 0.0)
                        aps = apspool.tile([128, TT], F32, tag="aps",
                                           name=f"aps{b}_{lh}_{qt}")

                        def attn_v(probs, kt, stop):
                            off = max(kt - 4 * qt, 0) * 128
                            nc.tensor.matmul(
                                aps[:, off:],
                                lhsT=v_sb[kt][:, lh * HD:(lh + 1) * HD],
                                rhs=probs[:, off:],
                                start=(kt == 0), stop=stop,
                                skip_group_check=True)

                        pend = []
                        for kt in range(n_kt):
                            d = kt - 4 * qt
                            off = max(d, 0) * 128
                            sps = pspool.tile([128, TT], F32, tag="ps",
                                              name=f"sps{b}_{lh}_{qt}_{kt}")
                            nc.tensor.matmul(
                                sps[:, off:],
                                lhsT=kT[lh][:, kt * 128:(kt + 1) * 128],
                                rhs=qT[lh][:, qt * TT + off:(qt + 1) * TT],
                                start=True, stop=True)
                            pump(1)
                            if len(pend) >= 4:
                                attn_v(*pend.pop(0), stop=False)
                            probs = prpool.tile([128, TT], BF16, tag="probs",
                                                name=f"pr{b}_{lh}_{qt}_{kt}")
                            nc.scalar.activation(probs[:, off:], sps[:, off:],
                                                 AF.Exp, scale=float(SCALE))
                            if d >= 0:
                                nc.vector.tensor_mul(
                                    probs[:, off:off + 128],
                                    probs[:, off:off + 128], mask_sb[:])
                            St = Sa if kt % 2 == 0 else Sb
                            if kt < 2:
                                nc.vector.tensor_copy(St[:, off:],
                                                      probs[:, off:])
                            else:
                                nc.vector.tensor_add(St[:, off:], St[:, off:],
                                                     probs[:, off:])
                            pend.append((probs, kt))
                        # merge parity partials BEFORE the AV drain: the
                        # drain covers the DVE latency so the rowsum matmul
                        # below doesn't stall the in-order PE queue.
                        nc.vector.tensor_add(Sa[:], Sa[:], Sb[:])
                        while pend:
                            attn_v(*pend.pop(0), stop=(len(pend) == 0))
                            pump(1)

                        # rowsum: contract+broadcast via all-ones matmul,
                        # reciprocal, normalize fused into the PSUM->SBUF
                        # copy.
                        rps = rpspool.tile([128, TT], F32, tag="rps",
                                           name=f"rs{b}{lh}{qt}")
                        nc.tensor.matmul(rps[:], lhsT=ones_sb[:], rhs=Sa[:],
                                         start=True, stop=True,
                                         skip_group_check=True)
                        recip = npool.tile([128, TT], F32, tag="rc",
                                           name=f"rc{b}{lh}{qt}")
                        nc.vector.reciprocal_approx_fast(out=recip[:],
                                                         in_=rps[:])
                        nc.vector.scalar_tensor_tensor(
                            at[lh][:, qt * TT:(qt + 1) * TT],
                            aps[:], 1.0, recip[:], ALU.mult, ALU.mult)
                        yield qt, lh

            def wo_blocks(b, st):
                """16 yields: partial output-projection blocks for batch b,
                token tile st; groups of 4 m-blocks share one SBUF tile and
                one store DMA."""
                at = attn_sb[b]
                for mg in range(DC // 4):
                    ost = ostpool.tile([128, 4 * TT], BF16, tag="ost",
                                       name=f"ost{b}{st}{mg}")
                    for mi in range(4):
                        m = mg * 4 + mi
                        ps = pspool.tile([128, TT], F32, tag="ps",
                                         name=f"pso{b}_{st}_{m}")
                        for ec in range(HL):
                            nc.tensor.matmul(
                                ps[:],
                                lhsT=wo_sb[:, ec * D + m * 128:
                                           ec * D + (m + 1) * 128],
                                rhs=at[ec][:, st * TT:(st + 1) * TT],
                                start=(ec == 0), stop=(ec == HL - 1),
                                skip_group_check=True)
                        if m % 2 == 0:
                            nc.scalar.copy(ost[:, mi * TT:(mi + 1) * TT],
                                           ps[:])
                        else:
                            nc.vector.tensor_copy(
                                ost[:, mi * TT:(mi + 1) * TT], ps[:])
                        if mi == 3:
                            nc.sync.dma_start(
                                out_ext.ap()[b * NT_B + st, mg], ost[:])
                        yield

            pending = []
            wo_defer = []

            def pump(n):
                while n > 0 and pending:
                    try:
                        next(pending[0])
                        n -= 1
                    except StopIteration:
                        pending.pop(0)

            def drive_attn(state, n):
                """Advance the (batch, generator) attention state by up to
                n groups, appending Wo generators as rounds complete."""
                bb, gen = state
                for _ in range(n):
                    r = next(gen, None)
                    if r is None:
                        return False
                    qt, lh = r
                    pump(2)
                    if lh == HL - 1:
                        g = wo_blocks(bb, qt)
                        # defer the last batch's Wo blocks so they fill the
                        # PE during the exp-bound attention tail.
                        if bb == B - 1:
                            wo_defer.append(g)
                        else:
                            pending.append(g)
                return True

            attn_state = None
            for b in range(B):
                i = 0
                gdone = 0
                self_state = None
                for _ in proj(b, pump):
                    i += 1
                    if i % 2 == 0:
                        if attn_state is not None:
                            if not drive_attn(attn_state, 1):
                                attn_state = None
                        if attn_state is None:
                            if self_state is None:
                                self_state = (b, attn_groups(b, *qkv[b],
                                                             pump))
                            if (gdone // 2) <= (i // 8) - 1:
                                if drive_attn(self_state, 1):
                                    gdone += 1
                if attn_state is not None:
                    while drive_attn(attn_state, 1):
                        pass
                attn_state = self_state if self_state is not None \
                    else (b, attn_groups(b, *qkv[b], pump))
            pending.extend(wo_defer)
            wo_defer.clear()
            while drive_attn(attn_state, 1):
                pass
            pending.extend(wo_defer)
            while pending:
                pump(16)

    nc.compile()
    return nc


def _prep_inputs(h, Wq, Wk, Wv, Wo, freqs_cos, freqs_sin):
    bf = ml_dtypes.bfloat16
    hT = np.asarray(h, np.float32).transpose(2, 0, 1).reshape(D, T)
    # hB[gt, p, dc*TT+tok] = hT[dc*128+p, gt*TT+tok]
    hB = np.ascontiguousarray(
        hT.reshape(DC, 128, NT, TT).transpose(2, 1, 0, 3)
          .reshape(NT, 128, DC * TT)).astype(bf)
    cosT = np.asarray(freqs_cos, np.float32).T
    sinT = np.asarray(freqs_sin, np.float32).T
    cosT = np.ascontiguousarray(np.concatenate([cosT, cosT], 0)).astype(bf)
    sinT = np.ascontiguousarray(np.concatenate([sinT, sinT], 0)).astype(bf)
    perm = np.concatenate([np.arange(0, HD, 2), np.arange(1, HD, 2)])
    p = np.arange(128)[:, None]
    j = np.arange(128)[None, :]
    mask128 = np.ascontiguousarray((j >= p).astype(np.float32)).astype(bf)

    Wq = np.asarray(Wq, np.float32); Wk = np.asarray(Wk, np.float32)
    Wv = np.asarray(Wv, np.float32); Wo = np.asarray(Wo, np.float32)

    def pack_w(wT):
        # wT: [D, E] -> [128, DC*E] with cols dc*E + e
        return np.ascontiguousarray(
            wT.reshape(DC, 128, E).transpose(1, 0, 2).reshape(128, DC * E)
        ).astype(bf)

    def pack_w_lh(wT):
        # wT: [D, E] -> [128, DC*E] with cols lh*(DC*HD) + dc*HD + hd
        return np.ascontiguousarray(
            wT.reshape(DC, 128, HL, HD).transpose(1, 2, 0, 3)
              .reshape(128, HL * DC * HD)).astype(bf)

    in_maps = []
    for g in range(NCORES):
        rows = slice(E * g, E * (g + 1))
        wq_s = Wq[rows, :].reshape(HL, HD, D)[:, perm, :].reshape(E, D)
        wk_s = Wk[rows, :].reshape(HL, HD, D)[:, perm, :].reshape(E, D)
        wv_s = Wv[rows, :]
        woT = Wo[:, rows].T                              # [E, D]
        woB = np.ascontiguousarray(
            woT.reshape(HL, 128, D).transpose(1, 0, 2).reshape(128, HL * D)
        ).astype(bf)
        in_maps.append({
            "hB": hB,
            "wqB": pack_w_lh(wq_s.T),
            "wkB": pack_w_lh(wk_s.T),
            "wvB": pack_w(wv_s.T),
            "woB": woB,
            "cosT": cosT,
            "sinT": sinT,
            "mask128": mask128,
        })
    return in_maps


def _run(in_maps, **kw):
    if "nc" not in _cache:
        _cache["nc"] = _build()
    return run_bass_kernel_spmd(_cache["nc"], in_maps,
                                core_ids=list(range(NCORES)), **kw)


def kernel(h, Wq, Wk, Wv, Wo, K_cache=None, V_cache=None,
           freqs_cos=None, freqs_sin=None, pos=0, **_ignored):
    assert int(pos) == 0
    in_maps = _prep_inputs(h, Wq, Wk, Wv, Wo, freqs_cos, freqs_sin)
    res = _run(in_maps)
    full = np.asarray(res.results[0]["out"], np.float32)
    for g in range(1, NCORES):
        full += np.asarray(res.results[g]["out"], np.float32)
    # out[b*4+st, mg, p, mi*512+tok] -> [B, S, D]
    full = full.reshape(B, NT_B, 4, 128, 4, TT).transpose(0, 1, 5, 2, 4, 3)
    return np.ascontiguousarray(full.reshape(B, S, D)).astype(np.float32)


# revision 22
# speedup vs baseline: 1.1846x; 1.1846x over previous
"""Trainium2 8-core tensor-parallel causal attention layer (prefill, pos=0),
collective-free.

Sharding: heads split across 8 cores (2 heads each). Each core computes QKV
projections + RoPE + causal attention for its 2 heads over all 4 batches,
then a PARTIAL output projection (its 256 rows of Wo's input dim, FULL output
dim D). The 8 partial outputs are summed host-side -- no on-device collective.

Per core:
  1. Q^T/K^T (head-dim-major) and V (token-major) projections from a
     host-relayouted bf16 copy of h (one contiguous 2MB block per 512-token
     tile -> ONE DMA per tile, double-buffered across tiles),
  2. RoPE via an even/odd head-dim permutation baked into Wq/Wk columns;
     the four projection chains of a token tile share one [128, 2048] bf16
     staging tile so the partition-half swap is 2 batched SBUF DMAs,
  3. causal attention in the transposed domain (scores^T = K^T_tile.T @ Q^T)
     with exact causal trim; exp on ACT; row sums via parity-partial DVE
     accumulation + one all-ones matmul (contract + partition-broadcast);
     reciprocal on DVE; normalization fused into the PSUM->SBUF copy,
  4. partial Wo blocks copied PSUM->SBUF in groups of 4 m-blocks and stored
     with ONE DMA per group to a group-contiguous DRAM layout.
Schedule: attention of batch b is interleaved under projection of batch b
(paced by tile readiness); leftover groups + Wo blocks fill the next batch's
projection.  Host-side: inputs transposed/sliced/cast bf16; partials summed.
"""

import numpy as np
import ml_dtypes

import concourse.bass as bass
import concourse.tile as tile
from concourse import bacc, mybir
from concourse.bass_utils import run_bass_kernel_spmd

BF16 = mybir.dt.bfloat16
F32 = mybir.dt.float32
AF = mybir.ActivationFunctionType
ALU = mybir.AluOpType

B, S, D = 4, 2048, 2048
H, HD = 16, 128
NCORES = 8
HL = H // NCORES          # heads per core = 2
E = HL * HD               # per-core qkv width = 256
T = B * S                 # tokens = 8192
TT = 512                  # token tile (free dim)
NT = T // TT              # global token tiles = 16
NT_B = S // TT            # token tiles per batch = 4
DC = D // 128             # contraction chunks = 16
SCALE = 1.0 / np.sqrt(HD)

_cache = {}


def _build():
    nc = bacc.Bacc("TRN2", target_bir_lowering=False, debug=False,
                   num_devices=NCORES)

    hB_ext = nc.dram_tensor("hB", [NT, 128, DC * TT], BF16,
                            kind="ExternalInput")
    wq_ext = nc.dram_tensor("wqB", [128, DC * E], BF16, kind="ExternalInput")
    wk_ext = nc.dram_tensor("wkB", [128, DC * E], BF16, kind="ExternalInput")
    wv_ext = nc.dram_tensor("wvB", [128, DC * E], BF16, kind="ExternalInput")
    wo_ext = nc.dram_tensor("woB", [128, HL * D], BF16, kind="ExternalInput")
    cos_ext = nc.dram_tensor("cosT", [128, S], BF16, kind="ExternalInput")
    sin_ext = nc.dram_tensor("sinT", [128, S], BF16, kind="ExternalInput")
    mask_ext = nc.dram_tensor("mask128", [128, 128], BF16,
                              kind="ExternalInput")
    out_ext = nc.dram_tensor("out", [16, 4, 128, 4 * TT], BF16,
                             kind="ExternalOutput")

    with tile.TileContext(nc) as tc:
        with (
            tc.tile_pool(name="weights", bufs=1) as wpool,
            tc.tile_pool(name="consts", bufs=1) as cpool,
            tc.tile_pool(name="ht", bufs=2) as htpool,
            tc.tile_pool(name="qkv", bufs=2) as qkvpool,
            tc.tile_pool(name="attn", bufs=2) as apool,
            tc.tile_pool(name="probs", bufs=6) as prpool,
            tc.tile_pool(name="ssum", bufs=3) as spool,
            tc.tile_pool(name="norm", bufs=3) as npool,
            tc.tile_pool(name="rope", bufs=2) as rbpool,
            tc.tile_pool(name="rtmp", bufs=4) as rpool,
            tc.tile_pool(name="ost", bufs=3) as ostpool,
            tc.tile_pool(name="ps", bufs=5, space="PSUM") as pspool,
            tc.tile_pool(name="psa", bufs=2, space="PSUM") as apspool,
            tc.tile_pool(name="psr", bufs=1, space="PSUM") as rpspool,
        ):
            htb = {}

            def load_ht(gt, chunks=1):
                t = htpool.tile([128, DC * TT], BF16, tag="ht",
                                name=f"ht{gt}")
                w = DC * TT // chunks
                for c in range(chunks):
                    nc.sync.dma_start(t[:, c * w:(c + 1) * w],
                                      hB_ext.ap()[gt][:, c * w:(c + 1) * w])
                htb[gt] = t

            # startup: first two token tiles on the sync queue (first one
            # chunked so the first matmul can start early); weights ordered
            # by first use on the gpsimd queue.
            # startup DMA order. sync ring (HWDGE, FIFO): the first chain's
            # weights, then the first token tile in 4 chunks, then the RoPE
            # tables (needed by ~25us), then the second token tile. gpsimd
            # ring concurrently: remaining Q/K weights, wv, mask, wo.
            HW = DC * HD          # per-head weight width = 2048
            wq_sb = wpool.tile([128, DC * E], BF16, tag="wq", name="wq")
            wk_sb = wpool.tile([128, DC * E], BF16, tag="wk", name="wk")
            nc.sync.dma_start(wq_sb[:, 0:HW], wq_ext.ap()[:, 0:HW])
            load_ht(0, chunks=4)
            cos_sb = cpool.tile([128, S], BF16, tag="cos", name="cos")
            nc.sync.dma_start(cos_sb[:], cos_ext.ap())
            sin_sb = cpool.tile([128, S], BF16, tag="sin", name="sin")
            nc.sync.dma_start(sin_sb[:], sin_ext.ap())
            load_ht(1)
            nc.gpsimd.dma_start(wq_sb[:, HW:2 * HW], wq_ext.ap()[:, HW:2 * HW])
            for lh in range(HL):
                nc.gpsimd.dma_start(
                    wk_sb[:, lh * HW:(lh + 1) * HW],
                    wk_ext.ap()[:, lh * HW:(lh + 1) * HW])
            wv_sb = wpool.tile([128, DC * E], BF16, tag="wv", name="wv")
            nc.gpsimd.dma_start(wv_sb[:], wv_ext.ap())
            mask_sb = cpool.tile([128, 128], BF16, tag="mask", name="mask")
            nc.gpsimd.dma_start(mask_sb[:], mask_ext.ap())
            wo_sb = wpool.tile([128, HL * D], BF16, tag="wo", name="wo")
            nc.gpsimd.dma_start(wo_sb[:], wo_ext.ap())
            ones_sb = cpool.tile([128, 128], BF16, tag="ones", name="ones")
            nc.vector.memset(ones_sb[:], 1.0)
            # HAM pre-warm: ~6us of dummy matmuls while the startup DMAs
            # land, so the real chains start at K=8/8 instead of paying the
            # half-clock window.
            warm_ps = rpspool.tile([128, TT], F32, tag="rps", name="warm")
            for _ in range(72):
                nc.tensor.matmul(warm_ps[:, 0:128], lhsT=ones_sb[:],
                                 rhs=ones_sb[:], start=True, stop=True,
                                 skip_group_check=True)

            # normalized attention outputs for this core's 2 heads, per batch
            attn_sb = {}

            qkv = {}

            def proj(b, pump=lambda n: None):
                """QKV projections + RoPE for batch b; yields after each
                chain / v-tile so attention can interleave (8 yields/tile)."""
                qT = [qkvpool.tile([HD, S], BF16, tag=f"qT{lh}",
                                   name=f"qT{lh}_{b}") for lh in range(HL)]
                kT = [qkvpool.tile([HD, S], BF16, tag=f"kT{lh}",
                                   name=f"kT{lh}_{b}") for lh in range(HL)]
                v_sb = [qkvpool.tile([128, E], BF16, tag=f"v{vt}",
                                     name=f"v{vt}_{b}")
                        for vt in range(S // 128)]
                qkv[b] = (qT, kT, v_sb)
                for tt in range(NT_B):
                    gt = NT_B * b + tt
                    ht = htb.pop(gt)
                    cs = cos_sb[:, tt * TT:(tt + 1) * TT]
                    sn = sin_sb[:, tt * TT:(tt + 1) * TT]
                    qsb = rbpool.tile([128, 4 * TT], BF16, tag="qsb",
                                      name=f"qsb{gt}")
                    qsw = rbpool.tile([128, 4 * TT], BF16, tag="qsw",
                                      name=f"qsw{gt}")
                    chains = [(wq_sb, qT, 0), (wq_sb, qT, 1),
                              (wk_sb, kT, 0), (wk_sb, kT, 1)]
                    for ci, (w_sb, dstT, lh) in enumerate(chains):
                        ps = pspool.tile([128, TT], F32, tag="ps",
                                         name=f"psp{gt}_{ci}")
                        for dc in range(DC):
                            nc.tensor.matmul(
                                ps[:],
                                lhsT=w_sb[:, lh * (DC * HD) + dc * HD:
                                          lh * (DC * HD) + (dc + 1) * HD],
                                rhs=ht[:, dc * TT:(dc + 1) * TT],
                                start=(dc == 0), stop=(dc == DC - 1))
                        # RoPE staging: psum rows 0:64 = even pairs (x0),
                        # 64:128 = odd (x1); ACT-copy PSUM->bf16.
                        nc.scalar.copy(qsb[:, ci * TT:(ci + 1) * TT], ps[:])
                        pump(1)
                        yield
                    # one batched partition-half swap for all 4 chains
                    nc.sync.dma_start(qsw[0:64, :], qsb[64:128, :])
                    nc.sync.dma_start(qsw[64:128, :], qsb[0:64, :])
                    # prefetch next token tile (slot of gt-1 just freed)
                    if gt + 2 < NT and gt + 2 not in htb:
                        load_ht(gt + 2)
                    # RoPE: m1 muls first (independent of the swap DMA) so
                    # the DVE queue isn't head-blocked on DMA latency.
                    m1s = []
                    for ci in range(4):
                        m1 = rpool.tile([128, TT], BF16, tag=f"m1_{ci % 2}",
                                        name=f"m1_{gt}{ci}")
                        nc.vector.tensor_mul(m1[:],
                                             qsb[:, ci * TT:(ci + 1) * TT],
                                             cs)
                        m1s.append(m1)
                    for ci, (w_sb, dstT, lh) in enumerate(chains):
                        dst = dstT[lh][:, tt * TT:(tt + 1) * TT]
                        m1 = m1s[ci]
                        m2 = rpool.tile([128, TT], BF16, tag=f"m2_{ci % 2}",
                                        name=f"m2_{gt}{ci}")
                        nc.vector.tensor_mul(m2[:],
                                             qsw[:, ci * TT:(ci + 1) * TT],
                                             sn)
                        nc.vector.tensor_sub(dst[0:64, :], m1[0:64, :],
                                             m2[0:64, :])
                        nc.vector.tensor_add(dst[64:128, :],
                                             m1[64:128, :], m2[64:128, :])
                    for vt in range(TT // 128):
                        ps = pspool.tile([128, E], F32, tag="ps",
                                         name=f"psv{gt}_{vt}")
                        for dc in range(DC):
                            nc.tensor.matmul(
                                ps[:],
                                lhsT=ht[:, dc * TT + vt * 128:
                                        dc * TT + (vt + 1) * 128],
                                rhs=wv_sb[:, dc * E:(dc + 1) * E],
                                start=(dc == 0), stop=(dc == DC - 1))
                        nc.scalar.copy(v_sb[tt * 4 + vt][:], ps[:])
                        pump(1)
                        yield

            def attn_groups(b, qT, kT, v_sb, pump=lambda n: None):
                """qt-outer/head-inner; yields (qt, lh) after each group so
                Wo blocks for tile qt can start once both heads are done."""
                at = [apool.tile([128, S], BF16, tag=f"at{lh}",
                                 name=f"at{lh}_{b}") for lh in range(HL)]
                attn_sb[b] = at
                for qt in range(NT_B):
                    for lh in range(HL):
                        n_kt = 4 * (qt + 1)
                        Sa = spool.tile([128, TT], BF16, tag="Sa",
                                        name=f"Sa{b}{lh}{qt}")
                        Sb = spool.tile([128, TT], BF16, tag="Sb",
                                        name=f"Sb{b}{lh}{qt}")
                        if qt == 0:
                            nc.vector.memset(Sb[:, 0:128], 0.0)
                        aps = apspool.tile([128, TT], F32, tag="aps",
                                           name=f"aps{b}_{lh}_{qt}")

                        def attn_v(probs, kt, stop):
                            off = max(kt - 4 * qt, 0) * 128
                            nc.tensor.matmul(
                                aps[:, off:],
                                lhsT=v_sb[kt][:, lh * HD:(lh + 1) * HD],
                                rhs=probs[:, off:],
                                start=(kt == 0), stop=stop,
                                skip_group_check=True)

                        pend = []
                        for kt in range(n_kt):
                            d = kt - 4 * qt
                            off = max(d, 0) * 128
                            sps = pspool.tile([128, TT], F32, tag="ps",
                                              name=f"sps{b}_{lh}_{qt}_{kt}")
                            nc.tensor.matmul(
                                sps[:, off:],
                                lhsT=kT[lh][:, kt * 128:(kt + 1) * 128],
                                rhs=qT[lh][:, qt * TT + off:(qt + 1) * TT],
                                start=True, stop=True)
                            pump(1)
                            if len(pend) >= 3:
                                attn_v(*pend.pop(0), stop=False)
                            probs = prpool.tile([128, TT], BF16, tag="probs",
                                                name=f"pr{b}_{lh}_{qt}_{kt}")
                            nc.scalar.activation(probs[:, off:], sps[:, off:],
                                                 AF.Exp, scale=float(SCALE))
                            if d >= 0:
                                nc.vector.tensor_mul(
                                    probs[:, off:off + 128],
                                    probs[:, off:off + 128], mask_sb[:])
                            St = Sa if kt % 2 == 0 else Sb
                            if kt < 2:
                                nc.vector.tensor_copy(St[:, off:],
                                                      probs[:, off:])
                            else:
                                nc.vector.tensor_add(St[:, off:], St[:, off:],
                                                     probs[:, off:])
                            pend.append((probs, kt))
                        # merge parity partials BEFORE the AV drain: the
                        # drain covers the DVE latency so the rowsum matmul
                        # below doesn't stall the in-order PE queue.
                        nc.vector.tensor_add(Sa[:], Sa[:], Sb[:])
                        while pend:
                            attn_v(*pend.pop(0), stop=(len(pend) == 0))
                            pump(1)

                        # rowsum: contract+broadcast via all-ones matmul,
                        # reciprocal, normalize fused into the PSUM->SBUF
                        # copy.
                        rps = rpspool.tile([128, TT], F32, tag="rps",
                                           name=f"rs{b}{lh}{qt}")
                        nc.tensor.matmul(rps[:], lhsT=ones_sb[:], rhs=Sa[:],
                                         start=True, stop=True,
                                         skip_group_check=True)
                        recip = npool.tile([128, TT], F32, tag="rc",
                                           name=f"rc{b}{lh}{qt}")
                        nc.vector.reciprocal_approx_fast(out=recip[:],
                                                         in_=rps[:])
                        nc.vector.scalar_tensor_tensor(
                            at[lh][:, qt * TT:(qt + 1) * TT],
                            aps[:], 1.0, recip[:], ALU.mult, ALU.mult)
                        yield qt, lh

            def wo_blocks(b, st):
                """16 yields: partial output-projection blocks for batch b,
                token tile st; groups of 4 m-blocks share one SBUF tile and
                one store DMA."""
                at = attn_sb[b]
                for mg in range(DC // 4):
                    ost = ostpool.tile([128, 4 * TT], BF16, tag="ost",
                                       name=f"ost{b}{st}{mg}")
                    for mi in range(4):
                        m = mg * 4 + mi
                        ps = pspool.tile([128, TT], F32, tag="ps",
                                         name=f"pso{b}_{st}_{m}")
                        for ec in range(HL):
                            nc.tensor.matmul(
                                ps[:],
                                lhsT=wo_sb[:, ec * D + m * 128:
                                           ec * D + (m + 1) * 128],
                                rhs=at[ec][:, st * TT:(st + 1) * TT],
                                start=(ec == 0), stop=(ec == HL - 1),
                                skip_group_check=True)
                        if m % 2 == 0:
                            nc.scalar.copy(ost[:, mi * TT:(mi + 1) * TT],
                                           ps[:])
                        else:
                            nc.vector.tensor_copy(
                                ost[:, mi * TT:(mi + 1) * TT], ps[:])
                        if mi == 3:
                            nc.sync.dma_start(
                                out_ext.ap()[b * NT_B + st, mg], ost[:])
                        yield

            pending = []
            wo_defer = []

            def pump(n):
                while n > 0 and pending:
                    try:
                        next(pending[0])
                        n -= 1
                    except StopIteration:
                        pending.pop(0)

            def drive_attn(state, n):
                """Advance the (batch, generator) attention state by up to
                n groups, appending Wo generators as rounds complete."""
                bb, gen = state
                for _ in range(n):
                    r = next(gen, None)
                    if r is None:
                        return False
                    qt, lh = r
                    pump(2)
                    if lh == HL - 1:
                        g = wo_blocks(bb, qt)
                        # defer the last batch's Wo blocks so they fill the
                        # PE during the exp-bound attention tail.
                        if bb == B - 1:
                            wo_defer.append(g)
                        else:
                            pending.append(g)
                return True

            attn_state = None
            for b in range(B):
                i = 0
                gdone = 0
                self_state = None
                for _ in proj(b, pump):
                    i += 1
                    if i % 2 == 0:
                        if attn_state is not None:
                            if not drive_attn(attn_state, 1):
                                attn_state = None
                        if attn_state is None:
                            if self_state is None:
                                self_state = (b, attn_groups(b, *qkv[b],
                                                             pump))
                            if (gdone // 2) <= (i // 8) - 1:
                                if drive_attn(self_state, 1):
                                    gdone += 1
                if attn_state is not None:
                    while drive_attn(attn_state, 1):
                        pass
                attn_state = self_state if self_state is not None \
                    else (b, attn_groups(b, *qkv[b], pump))
            pending.extend(wo_defer)
            wo_defer.clear()
            while drive_attn(attn_state, 1):
                pass
            pending.extend(wo_defer)
            while pending:
                pump(16)

    nc.compile()
    return nc


def _prep_inputs(h, Wq, Wk, Wv, Wo, freqs_cos, freqs_sin):
    bf = ml_dtypes.bfloat16
    hT = np.asarray(h, np.float32).transpose(2, 0, 1).reshape(D, T)
    # hB[gt, p, dc*TT+tok] = hT[dc*128+p, gt*TT+tok]
    hB = np.ascontiguousarray(
        hT.reshape(DC, 128, NT, TT).transpose(2, 1, 0, 3)
          .reshape(NT, 128, DC * TT)).astype(bf)
    cosT = np.asarray(freqs_cos, np.float32).T
    sinT = np.asarray(freqs_sin, np.float32).T
    cosT = np.ascontiguousarray(np.concatenate([cosT, cosT], 0)).astype(bf)
    sinT = np.ascontiguousarray(np.concatenate([sinT, sinT], 0)).astype(bf)
    perm = np.concatenate([np.arange(0, HD, 2), np.arange(1, HD, 2)])
    p = np.arange(128)[:, None]
    j = np.arange(128)[None, :]
    mask128 = np.ascontiguousarray((j >= p).astype(np.float32)).astype(bf)

    Wq = np.asarray(Wq, np.float32); Wk = np.asarray(Wk, np.float32)
    Wv = np.asarray(Wv, np.float32); Wo = np.asarray(Wo, np.float32)

    def pack_w(wT):
        # wT: [D, E] -> [128, DC*E] with cols dc*E + e
        return np.ascontiguousarray(
            wT.reshape(DC, 128, E).transpose(1, 0, 2).reshape(128, DC * E)
        ).astype(bf)

    def pack_w_lh(wT):
        # wT: [D, E] -> [128, DC*E] with cols lh*(DC*HD) + dc*HD + hd
        return np.ascontiguousarray(
            wT.reshape(DC, 128, HL, HD).transpose(1, 2, 0, 3)
              .reshape(128, HL * DC * HD)).astype(bf)

    in_maps = []
    for g in range(NCORES):
        rows = slice(E * g, E * (g + 1))
        wq_s = Wq[rows, :].reshape(HL, HD, D)[:, perm, :].reshape(E, D)
        wk_s = Wk[rows, :].reshape(HL, HD, D)[:, perm, :].reshape(E, D)
        wv_s = Wv[rows, :]
        woT = Wo[:, rows].T                              # [E, D]
        woB = np.ascontiguousarray(
            woT.reshape(HL, 128, D).transpose(1, 0, 2).reshape(128, HL * D)
        ).astype(bf)
        in_maps.append({
            "hB": hB,
            "wqB": pack_w_lh(wq_s.T),
            "wkB": pack_w_lh(wk_s.T),
            "wvB": pack_w(wv_s.T),
            "woB": woB,
            "cosT": cosT,
            "sinT": sinT,
            "mask128": mask128,
        })
    return in_maps


def _run(in_maps, **kw):
    if "nc" not in _cache:
        _cache["nc"] = _build()
    return run_bass_kernel_spmd(_cache["nc"], in_maps,
                                core_ids=list(range(NCORES)), **kw)


def kernel(h, Wq, Wk, Wv, Wo, K_cache=None, V_cache=None,
           freqs_cos=None, freqs_sin=None, pos=0, **_ignored):
    assert int(pos) == 0
    in_maps = _prep_inputs(h, Wq, Wk, Wv, Wo, freqs_cos, freqs_sin)
    res = _run(in_maps)
    full = np.asarray(res.results[0]["out"], np.float32)
    for g in range(1, NCORES):
        full += np.asarray(res.results[g]["out"], np.float32)
    # out[b*4+st, mg, p, mi*512+tok] -> [B, S, D]
    full = full.reshape(B, NT_B, 4, 128, 4, TT).transpose(0, 1, 5, 2, 4, 3)
    return np.ascontiguousarray(full.reshape(B, S, D)).astype(np.float32)


# revision 24
# speedup vs baseline: 1.1876x; 1.0025x over previous
"""Trainium2 8-core tensor-parallel causal attention layer (prefill, pos=0),
collective-free.

Sharding: heads split across 8 cores (2 heads each). Each core computes QKV
projections + RoPE + causal attention for its 2 heads over all 4 batches,
then a PARTIAL output projection (its 256 rows of Wo's input dim, FULL output
dim D). The 8 partial outputs are summed host-side -- no on-device collective.

Per core:
  1. Q^T/K^T (head-dim-major) and V (token-major) projections from a
     host-relayouted bf16 copy of h (one contiguous 2MB block per 512-token
     tile -> ONE DMA per tile, double-buffered across tiles),
  2. RoPE via an even/odd head-dim permutation baked into Wq/Wk columns;
     the four projection chains of a token tile share one [128, 2048] bf16
     staging tile so the partition-half swap is 2 batched SBUF DMAs,
  3. causal attention in the transposed domain (scores^T = K^T_tile.T @ Q^T)
     with exact causal trim; exp on ACT; row sums via parity-partial DVE
     accumulation + one all-ones matmul (contract + partition-broadcast);
     reciprocal on DVE; normalization fused into the PSUM->SBUF copy,
  4. partial Wo blocks copied PSUM->SBUF in groups of 4 m-blocks and stored
     with ONE DMA per group to a group-contiguous DRAM layout.
Schedule: attention of batch b is interleaved under projection of batch b
(paced by tile readiness); leftover groups + Wo blocks fill the next batch's
projection.  Host-side: inputs transposed/sliced/cast bf16; partials summed.
"""

import numpy as np
import ml_dtypes

import concourse.bass as bass
import concourse.tile as tile
from concourse import bacc, mybir
from concourse.bass_utils import run_bass_kernel_spmd

BF16 = mybir.dt.bfloat16
F32 = mybir.dt.float32
AF = mybir.ActivationFunctionType
ALU = mybir.AluOpType

B, S, D = 4, 2048, 2048
H, HD = 16, 128
NCORES = 8
HL = H // NCORES          # heads per core = 2
E = HL * HD               # per-core qkv width = 256
T = B * S                 # tokens = 8192
TT = 512                  # token tile (free dim)
NT = T // TT              # global token tiles = 16
NT_B = S // TT            # token tiles per batch = 4
DC = D // 128             # contraction chunks = 16
SCALE = 1.0 / np.sqrt(HD)

_cache = {}


def _build():
    nc = bacc.Bacc("TRN2", target_bir_lowering=False, debug=False,
                   num_devices=NCORES)

    hB_ext = nc.dram_tensor("hB", [NT, 128, DC * TT], BF16,
                            kind="ExternalInput")
    wq_ext = nc.dram_tensor("wqB", [128, DC * E], BF16, kind="ExternalInput")
    wk_ext = nc.dram_tensor("wkB", [128, DC * E], BF16, kind="ExternalInput")
    wv_ext = nc.dram_tensor("wvB", [128, DC * E], BF16, kind="ExternalInput")
    wo_ext = nc.dram_tensor("woB", [128, HL * D], BF16, kind="ExternalInput")
    cos_ext = nc.dram_tensor("cosT", [128, S], BF16, kind="ExternalInput")
    sin_ext = nc.dram_tensor("sinT", [128, S], BF16, kind="ExternalInput")
    mask_ext = nc.dram_tensor("mask128", [128, 128], BF16,
                              kind="ExternalInput")
    out_ext = nc.dram_tensor("out", [16, 4, 128, 4 * TT], BF16,
                             kind="ExternalOutput")

    with tile.TileContext(nc) as tc:
        with (
            tc.tile_pool(name="weights", bufs=1) as wpool,
            tc.tile_pool(name="consts", bufs=1) as cpool,
            tc.tile_pool(name="ht", bufs=2) as htpool,
            tc.tile_pool(name="qkv", bufs=2) as qkvpool,
            tc.tile_pool(name="attn", bufs=2) as apool,
            tc.tile_pool(name="probs", bufs=6) as prpool,
            tc.tile_pool(name="ssum", bufs=3) as spool,
            tc.tile_pool(name="norm", bufs=3) as npool,
            tc.tile_pool(name="rope", bufs=2) as rbpool,
            tc.tile_pool(name="rtmp", bufs=4) as rpool,
            tc.tile_pool(name="ost", bufs=3) as ostpool,
            tc.tile_pool(name="ps", bufs=5, space="PSUM") as pspool,
            tc.tile_pool(name="psa", bufs=2, space="PSUM") as apspool,
            tc.tile_pool(name="psr", bufs=1, space="PSUM") as rpspool,
        ):
            htb = {}

            def load_ht(gt, chunks=1):
                t = htpool.tile([128, DC * TT], BF16, tag="ht",
                                name=f"ht{gt}")
                w = DC * TT // chunks
                for c in range(chunks):
                    nc.sync.dma_start(t[:, c * w:(c + 1) * w],
                                      hB_ext.ap()[gt][:, c * w:(c + 1) * w])
                htb[gt] = t

            # startup: first two token tiles on the sync queue (first one
            # chunked so the first matmul can start early); weights ordered
            # by first use on the gpsimd queue.
            # startup DMA order. sync ring (HWDGE, FIFO): the first chain's
            # weights, then the first token tile in 4 chunks, then the RoPE
            # tables (needed by ~25us), then the second token tile. gpsimd
            # ring concurrently: remaining Q/K weights, wv, mask, wo.
            HW = DC * HD          # per-head weight width = 2048
            wq_sb = wpool.tile([128, DC * E], BF16, tag="wq", name="wq")
            wk_sb = wpool.tile([128, DC * E], BF16, tag="wk", name="wk")
            nc.sync.dma_start(wq_sb[:, 0:HW], wq_ext.ap()[:, 0:HW])
            load_ht(0, chunks=4)
            cos_sb = cpool.tile([128, S], BF16, tag="cos", name="cos")
            nc.sync.dma_start(cos_sb[:], cos_ext.ap())
            sin_sb = cpool.tile([128, S], BF16, tag="sin", name="sin")
            nc.sync.dma_start(sin_sb[:], sin_ext.ap())
            load_ht(1)
            nc.gpsimd.dma_start(wq_sb[:, HW:2 * HW], wq_ext.ap()[:, HW:2 * HW])
            for lh in range(HL):
                nc.gpsimd.dma_start(
                    wk_sb[:, lh * HW:(lh + 1) * HW],
                    wk_ext.ap()[:, lh * HW:(lh + 1) * HW])
            wv_sb = wpool.tile([128, DC * E], BF16, tag="wv", name="wv")
            nc.gpsimd.dma_start(wv_sb[:], wv_ext.ap())
            mask_sb = cpool.tile([128, 128], BF16, tag="mask", name="mask")
            nc.gpsimd.dma_start(mask_sb[:], mask_ext.ap())
            wo_sb = wpool.tile([128, HL * D], BF16, tag="wo", name="wo")
            nc.gpsimd.dma_start(wo_sb[:], wo_ext.ap())
            ones_sb = cpool.tile([128, 128], BF16, tag="ones", name="ones")
            nc.vector.memset(ones_sb[:], 1.0)
            # HAM pre-warm: ~6us of dummy matmuls while the startup DMAs
            # land, so the real chains start at K=8/8 instead of paying the
            # half-clock window.
            warm_ps = rpspool.tile([128, TT], F32, tag="rps", name="warm")
            for _ in range(72):
                nc.tensor.matmul(warm_ps[:, 0:128], lhsT=ones_sb[:],
                                 rhs=ones_sb[:], start=True, stop=True,
                                 skip_group_check=True)

            # normalized attention outputs for this core's 2 heads, per batch
            attn_sb = {}

            qkv = {}

            def proj(b, pump=lambda n: None):
                """QKV projections + RoPE for batch b; yields after each
                chain / v-tile so attention can interleave (8 yields/tile)."""
                qT = [qkvpool.tile([HD, S], BF16, tag=f"qT{lh}",
                                   name=f"qT{lh}_{b}") for lh in range(HL)]
                kT = [qkvpool.tile([HD, S], BF16, tag=f"kT{lh}",
                                   name=f"kT{lh}_{b}") for lh in range(HL)]
                v_sb = [qkvpool.tile([128, E], BF16, tag=f"v{vt}",
                                     name=f"v{vt}_{b}")
                        for vt in range(S // 128)]
                qkv[b] = (qT, kT, v_sb)
                for tt in range(NT_B):
                    gt = NT_B * b + tt
                    ht = htb.pop(gt)
                    cs = cos_sb[:, tt * TT:(tt + 1) * TT]
                    sn = sin_sb[:, tt * TT:(tt + 1) * TT]
                    qsb = rbpool.tile([128, 4 * TT], BF16, tag="qsb",
                                      name=f"qsb{gt}")
                    qsw = rbpool.tile([128, 4 * TT], BF16, tag="qsw",
                                      name=f"qsw{gt}")
                    chains = [(wq_sb, qT, 0), (wq_sb, qT, 1),
                              (wk_sb, kT, 0), (wk_sb, kT, 1)]
                    for ci, (w_sb, dstT, lh) in enumerate(chains):
                        ps = pspool.tile([128, TT], F32, tag="ps",
                                         name=f"psp{gt}_{ci}")
                        for dc in range(DC):
                            nc.tensor.matmul(
                                ps[:],
                                lhsT=w_sb[:, lh * (DC * HD) + dc * HD:
                                          lh * (DC * HD) + (dc + 1) * HD],
                                rhs=ht[:, dc * TT:(dc + 1) * TT],
                                start=(dc == 0), stop=(dc == DC - 1))
                        # RoPE staging: psum rows 0:64 = even pairs (x0),
                        # 64:128 = odd (x1); ACT-copy PSUM->bf16.
                        nc.scalar.copy(qsb[:, ci * TT:(ci + 1) * TT], ps[:])
                        if gt == 0:
                            # first tile's chains sputter on ht-chunk DMA
                            # arrival; keep the HAM activity window busy so
                            # they run at full clock.
                            for _ in range(8):
                                nc.tensor.matmul(warm_ps[:, 0:128],
                                                 lhsT=ones_sb[:],
                                                 rhs=ones_sb[:],
                                                 start=True, stop=True,
                                                 skip_group_check=True)
                        pump(1)
                        yield
                    # one batched partition-half swap for all 4 chains
                    nc.sync.dma_start(qsw[0:64, :], qsb[64:128, :])
                    nc.sync.dma_start(qsw[64:128, :], qsb[0:64, :])
                    # prefetch next token tile (slot of gt-1 just freed)
                    if gt + 2 < NT and gt + 2 not in htb:
                        load_ht(gt + 2)
                    # RoPE: m1 muls first (independent of the swap DMA) so
                    # the DVE queue isn't head-blocked on DMA latency.
                    m1s = []
                    for ci in range(4):
                        m1 = rpool.tile([128, TT], BF16, tag=f"m1_{ci % 2}",
                                        name=f"m1_{gt}{ci}")
                        nc.vector.tensor_mul(m1[:],
                                             qsb[:, ci * TT:(ci + 1) * TT],
                                             cs)
                        m1s.append(m1)
                    for ci, (w_sb, dstT, lh) in enumerate(chains):
                        dst = dstT[lh][:, tt * TT:(tt + 1) * TT]
                        m1 = m1s[ci]
                        m2 = rpool.tile([128, TT], BF16, tag=f"m2_{ci % 2}",
                                        name=f"m2_{gt}{ci}")
                        nc.vector.tensor_mul(m2[:],
                                             qsw[:, ci * TT:(ci + 1) * TT],
                                             sn)
                        nc.vector.tensor_sub(dst[0:64, :], m1[0:64, :],
                                             m2[0:64, :])
                        nc.vector.tensor_add(dst[64:128, :],
                                             m1[64:128, :], m2[64:128, :])
                    for vt in range(TT // 128):
                        ps = pspool.tile([128, E], F32, tag="ps",
                                         name=f"psv{gt}_{vt}")
                        for dc in range(DC):
                            nc.tensor.matmul(
                                ps[:],
                                lhsT=ht[:, dc * TT + vt * 128:
                                        dc * TT + (vt + 1) * 128],
                                rhs=wv_sb[:, dc * E:(dc + 1) * E],
                                start=(dc == 0), stop=(dc == DC - 1))
                        nc.scalar.copy(v_sb[tt * 4 + vt][:], ps[:])
                        pump(1)
                        yield

            def attn_groups(b, qT, kT, v_sb, pump=lambda n: None):
                """qt-outer/head-inner; yields (qt, lh) after each group so
                Wo blocks for tile qt can start once both heads are done."""
                at = [apool.tile([128, S], BF16, tag=f"at{lh}",
                                 name=f"at{lh}_{b}") for lh in range(HL)]
                attn_sb[b] = at
                for qt in range(NT_B):
                    for lh in range(HL):
                        n_kt = 4 * (qt + 1)
                        Sa = spool.tile([128, TT], BF16, tag="Sa",
                                        name=f"Sa{b}{lh}{qt}")
                        Sb = spool.tile([128, TT], BF16, tag="Sb",
                                        name=f"Sb{b}{lh}{qt}")
                        if qt == 0:
                            nc.vector.memset(Sb[:, 0:128], 0.0)
                        aps = apspool.tile([128, TT], F32, tag="aps",
                                           name=f"aps{b}_{lh}_{qt}")

                        def attn_v(probs, kt, stop):
                            off = max(kt - 4 * qt, 0) * 128
                            nc.tensor.matmul(
                                aps[:, off:],
                                lhsT=v_sb[kt][:, lh * HD:(lh + 1) * HD],
                                rhs=probs[:, off:],
                                start=(kt == 0), stop=stop,
                                skip_group_check=True)

                        pend = []
                        for kt in range(n_kt):
                            d = kt - 4 * qt
                            off = max(d, 0) * 128
                            sps = pspool.tile([128, TT], F32, tag="ps",
                                              name=f"sps{b}_{lh}_{qt}_{kt}")
                            nc.tensor.matmul(
                                sps[:, off:],
                                lhsT=kT[lh][:, kt * 128:(kt + 1) * 128],
                                rhs=qT[lh][:, qt * TT + off:(qt + 1) * TT],
                                start=True, stop=True)
                            pump(1)
                            if len(pend) >= 3:
                                attn_v(*pend.pop(0), stop=False)
                            probs = prpool.tile([128, TT], BF16, tag="probs",
                                                name=f"pr{b}_{lh}_{qt}_{kt}")
                            nc.scalar.activation(probs[:, off:], sps[:, off:],
                                                 AF.Exp, scale=float(SCALE))
                            if d >= 0:
                                nc.vector.tensor_mul(
                                    probs[:, off:off + 128],
                                    probs[:, off:off + 128], mask_sb[:])
                            St = Sa if kt % 2 == 0 else Sb
                            if kt < 2:
                                nc.vector.tensor_copy(St[:, off:],
                                                      probs[:, off:])
                            else:
                                nc.vector.tensor_add(St[:, off:], St[:, off:],
                                                     probs[:, off:])
                            pend.append((probs, kt))
                        # merge parity partials BEFORE the AV drain: the
                        # drain covers the DVE latency so the rowsum matmul
                        # below doesn't stall the in-order PE queue.
                        nc.vector.tensor_add(Sa[:], Sa[:], Sb[:])
                        while pend:
                            attn_v(*pend.pop(0), stop=(len(pend) == 0))
                            pump(1)

                        # rowsum: contract+broadcast via all-ones matmul,
                        # reciprocal, normalize fused into the PSUM->SBUF
                        # copy.
                        rps = rpspool.tile([128, TT], F32, tag="rps",
                                           name=f"rs{b}{lh}{qt}")
                        nc.tensor.matmul(rps[:], lhsT=ones_sb[:], rhs=Sa[:],
                                         start=True, stop=True,
                                         skip_group_check=True)
                        recip = npool.tile([128, TT], F32, tag="rc",
                                           name=f"rc{b}{lh}{qt}")
                        nc.vector.reciprocal_approx_fast(out=recip[:],
                                                         in_=rps[:])
                        nc.vector.scalar_tensor_tensor(
                            at[lh][:, qt * TT:(qt + 1) * TT],
                            aps[:], 1.0, recip[:], ALU.mult, ALU.mult)
                        yield qt, lh

            def wo_blocks(b, st):
                """16 yields: partial output-projection blocks for batch b,
                token tile st; groups of 4 m-blocks share one SBUF tile and
                one store DMA."""
                at = attn_sb[b]
                for mg in range(DC // 4):
                    ost = ostpool.tile([128, 4 * TT], BF16, tag="ost",
                                       name=f"ost{b}{st}{mg}")
                    for mi in range(4):
                        m = mg * 4 + mi
                        ps = pspool.tile([128, TT], F32, tag="ps",
                                         name=f"pso{b}_{st}_{m}")
                        for ec in range(HL):
                            nc.tensor.matmul(
                                ps[:],
                                lhsT=wo_sb[:, ec * D + m * 128:
                                           ec * D + (m + 1) * 128],
                                rhs=at[ec][:, st * TT:(st + 1) * TT],
                                start=(ec == 0), stop=(ec == HL - 1),
                                skip_group_check=True)
                        if m % 2 == 0:
                            nc.scalar.copy(ost[:, mi * TT:(mi + 1) * TT],
                                           ps[:])
                        else:
                            nc.vector.tensor_copy(
                                ost[:, mi * TT:(mi + 1) * TT], ps[:])
                        if mi == 3:
                            nc.sync.dma_start(
                                out_ext.ap()[b * NT_B + st, mg], ost[:])
                        yield

            pending = []
            wo_defer = []

            def pump(n):
                while n > 0 and pending:
                    try:
                        next(pending[0])
                        n -= 1
                    except StopIteration:
                        pending.pop(0)

            def drive_attn(state, n):
                """Advance the (batch, generator) attention state by up to
                n groups, appending Wo generators as rounds complete."""
                bb, gen = state
                for _ in range(n):
                    r = next(gen, None)
                    if r is None:
                        return False
                    qt, lh = r
                    pump(2)
                    if lh == HL - 1:
                        g = wo_blocks(bb, qt)
                        # defer the last batch's Wo blocks so they fill the
                        # PE during the exp-bound attention tail.
                        if bb == B - 1:
                            wo_defer.append(g)
                        else:
                            pending.append(g)
                        # release them as the final (biggest) groups start,
                        # so their pumps interleave the blocks into the
                        # exp-paced AV stalls instead of queuing them all
                        # behind the last rowsum.
                        if bb == B - 1 and qt == NT_B - 2:
                            pending.extend(wo_defer)
                            wo_defer.clear()
                return True

            attn_state = None
            for b in range(B):
                i = 0
                gdone = 0
                self_state = None
                for _ in proj(b, pump):
                    i += 1
                    if i % 2 == 0:
                        if attn_state is not None:
                            if not drive_attn(attn_state, 1):
                                attn_state = None
                        if attn_state is None:
                            if self_state is None:
                                self_state = (b, attn_groups(b, *qkv[b],
                                                             pump))
                            if (gdone // 2) <= (i // 8) - 1:
                                if drive_attn(self_state, 1):
                                    gdone += 1
                if attn_state is not None:
                    while drive_attn(attn_state, 1):
                        pass
                attn_state = self_state if self_state is not None \
                    else (b, attn_groups(b, *qkv[b], pump))
            pending.extend(wo_defer)
            wo_defer.clear()
            while drive_attn(attn_state, 1):
                pass
            pending.extend(wo_defer)
            while pending:
                pump(16)

    nc.compile()
    return nc


def _prep_inputs(h, Wq, Wk, Wv, Wo, freqs_cos, freqs_sin):
    bf = ml_dtypes.bfloat16
    hT = np.asarray(h, np.float32).transpose(2, 0, 1).reshape(D, T)
    # hB[gt, p, dc*TT+tok] = hT[dc*128+p, gt*TT+tok]
    hB = np.ascontiguousarray(
        hT.reshape(DC, 128, NT, TT).transpose(2, 1, 0, 3)
          .reshape(NT, 128, DC * TT)).astype(bf)
    cosT = np.asarray(freqs_cos, np.float32).T
    sinT = np.asarray(freqs_sin, np.float32).T
    cosT = np.ascontiguousarray(np.concatenate([cosT, cosT], 0)).astype(bf)
    sinT = np.ascontiguousarray(np.concatenate([sinT, sinT], 0)).astype(bf)
    perm = np.concatenate([np.arange(0, HD, 2), np.arange(1, HD, 2)])
    p = np.arange(128)[:, None]
    j = np.arange(128)[None, :]
    mask128 = np.ascontiguousarray((j >= p).astype(np.float32)).astype(bf)

    Wq = np.asarray(Wq, np.float32); Wk = np.asarray(Wk, np.float32)
    Wv = np.asarray(Wv, np.float32); Wo = np.asarray(Wo, np.float32)

    def pack_w(wT):
        # wT: [D, E] -> [128, DC*E] with cols dc*E + e
        return np.ascontiguousarray(
            wT.reshape(DC, 128, E).transpose(1, 0, 2).reshape(128, DC * E)
        ).astype(bf)

    def pack_w_lh(wT):
        # wT: [D, E] -> [128, DC*E] with cols lh*(DC*HD) + dc*HD + hd
        return np.ascontiguousarray(
            wT.reshape(DC, 128, HL, HD).transpose(1, 2, 0, 3)
              .reshape(128, HL * DC * HD)).astype(bf)

    in_maps = []
    for g in range(NCORES):
        rows = slice(E * g, E * (g + 1))
        wq_s = Wq[rows, :].reshape(HL, HD, D)[:, perm, :].reshape(E, D)
        wk_s = Wk[rows, :].reshape(HL, HD, D)[:, perm, :].reshape(E, D)
        wv_s = Wv[rows, :]
        woT = Wo[:, rows].T                              # [E, D]
        woB = np.ascontiguousarray(
            woT.reshape(HL, 128, D).transpose(1, 0, 2).reshape(128, HL * D)
        ).astype(bf)
        in_maps.append({
            "hB": hB,
            "wqB": pack_w_lh(wq_s.T),
            "wkB": pack_w_lh(wk_s.T),
            "wvB": pack_w(wv_s.T),
            "woB": woB,
            "cosT": cosT,
            "sinT": sinT,
            "mask128": mask128,
        })
    return in_maps


def _run(in_maps, **kw):
    if "nc" not in _cache:
        _cache["nc"] = _build()
    return run_bass_kernel_spmd(_cache["nc"], in_maps,
                                core_ids=list(range(NCORES)), **kw)


def kernel(h, Wq, Wk, Wv, Wo, K_cache=None, V_cache=None,
           freqs_cos=None, freqs_sin=None, pos=0, **_ignored):
    assert int(pos) == 0
    in_maps = _prep_inputs(h, Wq, Wk, Wv, Wo, freqs_cos, freqs_sin)
    res = _run(in_maps)
    full = np.asarray(res.results[0]["out"], np.float32)
    for g in range(1, NCORES):
        full += np.asarray(res.results[g]["out"], np.float32)
    # out[b*4+st, mg, p, mi*512+tok] -> [B, S, D]
    full = full.reshape(B, NT_B, 4, 128, 4, TT).transpose(0, 1, 5, 2, 4, 3)
    return np.ascontiguousarray(full.reshape(B, S, D)).astype(np.float32)
